# revision 59
# baseline (speedup 1.0000x reference)
"""Trainium2 Bass kernel for nn_Attention_Param_sharing_Kv_sharing.

Reference computation (per batch b, with x_b = x[b] viewed as [C=256, N=4096]):
    K   = w_qk' @ x_b + t_qk                  [16, N]    (BN folded into w', t)
    S   = K^T K                               [N, N]     (q == k shared -> symmetric)
    P   = exp(S)        (no max-subtraction; |S| < ~40 so fp32 exp is safe)
    r   = row sums of P = column sums of P    (symmetry)
    XXu^T[c,n] = sum_m V[c,m] P[m,n]          (= (attn @ V) * r, pre-normalized)
    out = (w_p' @ relu(XXu^T) + t_p (x) r) * (1/r)       [256, N]

Sharding: 8 cores = 4 batches x 2 column-halves of N.  The host permutes the
spatial axis per core so each core's own 2048 columns come first (attention
is permutation-equivariant over m when K and V are permuted together, and r
is permutation-invariant), which keeps the device program SPMD-uniform.

The device returns the unnormalized projection O_u = w_p' @ relu(XXu^T)
plus per-ROW sums R[m] = sum_n P[m, n-half].  R comes for free from the
Scalar engine's accumulate port (`accum_out`) on the exp instruction itself:
the n-loop runs in 1024-wide superblocks so each exp call covers exactly one
m-tile, making the per-instruction accumulator exactly R[m-tile] for that
superblock.  Because P is symmetric, the column sums r[n] are recovered on
the host as R_half0[n] + R_half1[n] from a batch's two half-slabs, and the
final (O_u + t_p (x) r) / r = O_u/r + t_p is a trivial elementwise host
epilogue (same class of host work as the BN folding / permutation already
done in make_in_maps).  This removes the 128 PE row-sum matmuls, the rank-1
t_p (x) r PSUM updates and the whole reciprocal/broadcast chain of the
previous version.

Symmetry of P means the P tiles computed in [m-partition, n-free] layout are
directly the P^T operand needed by the attn@V matmul -- no transposes.  P
tiles are consumed by attn@V immediately after exp, so only a small
round-robin window of them lives in SBUF.

Diagonal-block dedup: within a core's own half, the score block
P[1024:2048, 0:1024] (m-tiles 8-15 of superblock 0) is the transpose of
P[0:1024, 1024:2048] (m-tiles 0-7 of superblock 1), so the latter's S
matmuls and exps -- 8 of 64 exp calls, 12.5% of the binding Scalar-engine
stream -- are skipped.  The source tiles are kept alive in a dedicated
SBUF pool and rebuilt into a pT buffer by PE transpose instructions
(dripped one 4-transpose half-op per exp shadow through superblock 1),
whose attn@V contributions are appended in the last four rounds.  The
skipped tiles' row-sum (R) contributions are the PARTITION sums of the
source tiles, accumulated by ones-vector matmuls in a single PSUM bank
(two sequential 8-matmul chains) and shipped to the host as the separate
r2 output.  PSUM budget: S double-buffer 4 banks + attn@V accumulators 2
+ projection/r2 1 + transpose staging 1 = 8.
"""

import numpy as np
import ml_dtypes

import concourse.bass as bass
import concourse.mybir as mybir
import concourse.tile as tile
from concourse import bacc
from concourse.bass import ts

F32 = mybir.dt.float32
F32R = mybir.dt.float32r
BF16 = mybir.dt.bfloat16

N_CORES = 8
B, C, H, W = 4, 256, 64, 64
N = H * W            # 4096
KD = 16              # qk dim
DH = 128             # value channels
EPS = 1e-5

NSH = N // 2         # 2048 n-columns per core
NSB = 1024           # n-superblock width (one exp call = one m-tile x NSB)
NSBLOCKS = NSH // NSB  # 2
NBLK = 512           # psum-bank chunk
MT = N // 128        # 32 m-tiles

_CACHE = {}


def _prologue(nc, pools, dram, probe=None, rep=0):
    """Input tiles + chunked DMAs + the first two K-projection chunks.

    In steady state this is emitted near the END of the previous rep (via
    the prefetch hook), so the next rep's first S matmul finds k_sb chunks
    0-1 already materialized and the ACT boundary gap shrinks to ~1us.
    k_sb is double-buffered so the write doesn't wait on the previous rep's
    final S reads.
    """
    const, work, outp, pgrp, psrc, ps_s, ps_xx, ps_pj, ps_t = pools
    (xf_d, xb_d, wqkT_d, wvT_d, wpT_d, tqk_d, tv_d, out_d, r_d,
     ident_d, r2_d) = dram

    xf = const.tile([128, 2, N], F32R, tag="xf")
    xb = const.tile([128, 2, N], BF16, tag="xb")
    wqkT = const.tile([128, 2, 128], F32R, tag="wqkT")
    wvT = const.tile([128, 2, DH], BF16, tag="wvT")
    wpT = const.tile([128, 2, 128], F32R, tag="wpT")
    tqk = const.tile([128, 1], F32, tag="tqk")
    tvb = const.tile([128, DH], F32, tag="tvb")

    def chunk_dma(dst, src_d, i, w):
        nc.sync.dma_start(
            out=dst[:, :, ts(i, w)],
            in_=src_d.ap()[:, :, ts(i, w)],
        )

    nc.sync.dma_start(out=wqkT, in_=wqkT_d.ap())
    nc.sync.dma_start(out=tqk, in_=tqk_d.ap())
    chunk_dma(xf, xf_d, 0, 512)   # first eighth: unblocks K-proj chunk 0
    chunk_dma(xf, xf_d, 1, 512)
    nc.sync.dma_start(out=wvT, in_=wvT_d.ap())
    nc.sync.dma_start(
        out=tvb, in_=bass.AP(tensor=tv_d, offset=0, ap=[[0, 128], [1, DH]])
    )
    chunk_dma(xb, xb_d, 0, 1024)
    chunk_dma(xf, xf_d, 1, 1024)
    chunk_dma(xf, xf_d, 2, 1024)
    chunk_dma(xf, xf_d, 3, 1024)
    chunk_dma(xb, xb_d, 1, 1024)
    chunk_dma(xb, xb_d, 2, 1024)
    chunk_dma(xb, xb_d, 3, 1024)
    nc.sync.dma_start(out=wpT, in_=wpT_d.ap())
    ident = const.tile([128, 128], BF16, tag="ident")
    nc.sync.dma_start(out=ident, in_=ident_d.ap())
    ones_bf = const.tile([128, 1], BF16, tag="ones_bf")
    nc.vector.memset(ones_bf, 1.0)
    # R[m] accumulators: one [128,1] column per (m-tile, superblock);
    # double-buffered so the first accum doesn't wait the previous rep's
    # deferred R write-out DMA
    nb = 1 if probe == "prebuf" else 2
    r_sb = const.tile([128, MT, NSBLOCKS], F32, tag="r_sb", bufs=nb)
    if probe == "no_accum":  # timing probe: r_sb otherwise unwritten
        nc.vector.memset(r_sb, 1.0)
    if rep == 0:
        # dummy exp: loads the ACT exp table set during the prologue instead
        # of stalling the first real exp call ~2.7us
        warm_sb = work.tile([1, 1], F32, tag="warm")
        nc.scalar.activation(
            out=warm_sb, in_=tqk[0:1, 0:1],
            func=mybir.ActivationFunctionType.Exp,
        )
        # warm-up matmuls: start the PE p-state ramp during the input DMA
        # so the first K-projection doesn't run at the cold clock
        for i in range(2):
            wps_full = ps_pj.tile([128, NBLK], F32, tag="pj", name="wps")
            nc.tensor.matmul(
                wps_full[:, 0:64], wqkT[:, 0, :], wqkT[:, 1, 0:64],
                start=True, stop=True,
            )

    # K projection (replicated 4x across 32-row groups for S packing):
    # k_sb rows 32g+d (d<16) hold K[d, :]; rows 32g+16.. are zero.
    # 512-column chunks; chunks 0-1 here, the rest interleaved into
    # superblock 0's round loop so the in-order PE stream never parks
    # behind a late xf DMA.
    k_sb = const.tile([128, N], F32R, tag="k_sb", bufs=nb)

    def emit_kproj(i):
        kps_full = ps_s.tile([128, NSB], F32, tag="s", name="kps")
        kps = kps_full[:, 0:NBLK]
        for cb in range(2):
            nc.tensor.matmul(
                kps,
                wqkT[:, cb, :],
                xf[:, cb, ts(i, NBLK)],
                start=(cb == 0),
                stop=(cb == 1),
            )
        nc.vector.tensor_scalar(
            out=k_sb[:, ts(i, NBLK)],
            in0=kps,
            scalar1=tqk,
            scalar2=None,
            op0=mybir.AluOpType.add,
        )

    for i in range(2):
        emit_kproj(i)

    return dict(xf=xf, xb=xb, wqkT=wqkT, wvT=wvT, wpT=wpT, tqk=tqk,
                tvb=tvb, r_sb=r_sb, k_sb=k_sb, emit_kproj=emit_kproj,
                ident=ident, ones_bf=ones_bf)


def _emit(nc, pools, dram, ctx, pack_s=True, probe=None, rep=0,
          deferred=None, hook=None):
    const, work, outp, pgrp, psrc, ps_s, ps_xx, ps_pj, ps_t = pools
    (xf_d, xb_d, wqkT_d, wvT_d, wpT_d, tqk_d, tv_d, out_d, r_d,
     ident_d, r2_d) = dram
    xb, wvT, wpT, tvb = ctx["xb"], ctx["wvT"], ctx["wpT"], ctx["tvb"]
    r_sb, k_sb, emit_kproj = ctx["r_sb"], ctx["k_sb"], ctx["emit_kproj"]

    # ---- V^T: VT[m, c] = sum_C x[C, m] wv'[c, C] + tv  -> bf16 ----
    # All of V^T is emitted inside superblock 0's round loop (one round of
    # lookahead) so the scalar engine is already busy with exp while the
    # small projections run.
    vt_sb = const.tile([128, MT, DH], BF16, tag="vt_sb")

    def emit_vt(mi):
        vps_full = ps_pj.tile([128, NBLK], F32, tag="pj", name="vps")
        vps = vps_full[:, 0:DH]
        for cb in range(2):
            nc.tensor.matmul(
                vps,
                xb[:, cb, ts(mi, 128)],
                wvT[:, cb, :],
                start=(cb == 0),
                stop=(cb == 1),
            )
        nc.vector.tensor_add(vt_sb[:, mi, :], vps, tvb)

    # ---- main loop over this core's n-superblocks ----
    # The epilogue for superblock J-1 is software-pipelined into J's round
    # loop so its DVE/PE work overlaps ACT's exp stream.
    def epilogue_pieces(st, tail=False):
        """Four pieces, one per (chunk, out-half), to be spread across
        successive exp shadows so no single shadow overflows."""
        J, xxp = st["J"], st["xx"]
        relus = {}

        def make(c, h2):
            def piece():
                if h2 == 0:
                    relu_sb = work.tile([128, NBLK], F32R, tag="relu")
                    if tail:
                        # ACT is idle once the exp stream ends
                        nc.scalar.activation(
                            out=relu_sb,
                            in_=xxp[c],
                            func=mybir.ActivationFunctionType.Relu,
                        )
                    else:
                        nc.vector.tensor_scalar(
                            out=relu_sb,
                            in0=xxp[c],
                            scalar1=0.0,
                            scalar2=None,
                            op0=mybir.AluOpType.max,
                        )
                    relus[c] = relu_sb
                pjps = ps_pj.tile([128, NBLK], F32, tag="pj")
                nc.tensor.matmul(
                    pjps, wpT[:, h2, :], relus[c], start=True, stop=True
                )
                o_sb = outp.tile([128, NBLK], F32, tag="o")
                nc.vector.tensor_copy(o_sb, pjps)
                nc.sync.dma_start(
                    out=out_d[h2, :, ts(J * 2 + c, NBLK)], in_=o_sb
                )
            return piece

        # relus (h2==0 pieces) first: they free the xx psum slots that the
        # next superblock's attn@V accumulation is waiting to reuse; pieces
        # are kind-tagged so PSUM-hungry pj pieces can be held back while
        # the r2 row-sum accumulators occupy the pj banks
        return [("relu", make(0, 0)), ("relu", make(1, 0)),
                ("pj", make(0, 1)), ("pj", make(1, 1))]

    # queue of deferred epilogue pieces to drip into upcoming exp shadows
    pieces = list(deferred) if deferred else []
    dedup = probe != "nodedup"

    src_tiles = []  # J0 round-tiles holding m-tiles 8..15 (diag sources)
    prev = None
    for J in range(NSBLOCKS):
        xxA = ps_xx.tile([128, NBLK], F32, tag="xx", name="xxA")
        xxB = ps_xx.tile([128, NBLK], F32, tag="xx", name="xxB")
        xxp = (xxA, xxB)
        mi0 = 8 if (dedup and J == 1) else 0  # first exp'd m-tile of this J

        # attn@V runs one m-tile behind exp, so the PE work gating the next
        # exp (its 2 S matmuls) plus the deferred attn@V of the previous
        # m-tile both fit inside the ACT shadow of the current exp.
        pend = None  # (p_sb, q, mi) owing its attn@V

        def emit_attnv(ent, mi0=None):
            pp, q, mi = ent
            if probe == "half_exp":
                q = 0  # odd p tiles are never exp'd in this timing probe
            for c in range(2):
                nc.tensor.matmul(
                    xxp[c],
                    vt_sb[:, mi, :],
                    pp[:, q, ts(c, NBLK)],
                    start=(mi == mi0),
                    stop=(mi == MT - 1),
                )

        if dedup and J == 1:
            # transposed-diag buffer for the 8 skipped m-tiles of J=1
            pT_sb = const.tile([128, 8, NSB], BF16, tag="pT")
            r2_sb = outp.tile([1, NSB], F32, tag="r2")
            r2st = {"ps": None}

            def src_ap(k):
                return src_tiles[k // 2][:, k % 2, :]

            def emit_xq(s):
                """Half-op s: 4 transposes of diag source k=s//2 (column
                half s%2) into the dedicated 1-bank tps slot + evacuation
                into pT, plus one r2 row-sum matmul (c=0 chain over slots
                0..7, c=1 chain over 8..15, sharing the single pj bank)."""
                k, half = s // 2, s % 2
                src = src_ap(k)
                tps = ps_t.tile([128, 4, 128], BF16, tag="t", name="tps")
                for j2 in range(4):
                    j = 4 * half + j2
                    nc.tensor.transpose(
                        tps[:, j2, :], src[:, ts(j, 128)], ctx["ident"]
                    )
                with nc.allow_low_precision(reason="bf16 P transpose"):
                    nc.vector.tensor_copy(
                        pT_sb[:, 4 * half:4 * half + 4, ts(k, 128)],
                        tps,
                    )
                c, kk = s // 8, s % 8
                if kk == 0:
                    if c == 1:  # c=0 chain done: copy it out, free the bank
                        nc.vector.tensor_copy(
                            r2_sb[:, ts(0, NBLK)], r2st["ps"])
                    r2f = ps_pj.tile([128, NBLK], F32, tag="pj", name="r2ps")
                    r2st["ps"] = r2f[0:1, :]
                nc.tensor.matmul(
                    r2st["ps"], ctx["ones_bf"], src_ap(kk)[:, ts(c, NBLK)],
                    start=(kk == 0), stop=(kk == 7),
                )

        rounds = range(4, MT // 2) if (dedup and J == 1) else range(MT // 2)
        for t in rounds:  # rounds of 2 m-tiles
            use_src = dedup and J == 0 and 4 <= t < 8
            pool = psrc if use_src else pgrp
            p_sb = pool.tile([128, 2, NSB], BF16,
                             tag="psrc" if use_src else "p")
            if use_src:
                src_tiles.append(p_sb)
            # Each exp is gated by only its own m-tile's 2 S matmuls; the
            # 4 S matmuls of a round are packed into distinct 32-row PE
            # groups.  exp's ACT accumulator gives R[m-tile] for free.
            for q in range(2):
                mi = 2 * t + q
                s_ps = ps_s.tile([128, NSB], F32, tag="s")
                for c in range(2):
                    g = 32 * (2 * q + c) if pack_s else 0
                    nc.tensor.matmul(
                        s_ps[:, ts(c, NBLK)],
                        k_sb[g:g + KD, ts(mi, 128)],
                        k_sb[g:g + KD, ts(J * 2 + c, NBLK)],
                        start=True,
                        stop=True,
                        tile_position=(g, 0),
                    )
                # drip one queued epilogue piece (previous superblock or
                # previous rep's tail) into this exp's shadow (q==1: the
                # V^T lookahead occupies the q==0 shadow).  pj pieces are
                # held while the r2 accumulators own the pj banks.
                allow_pj = not (dedup and J == 1)
                if probe == "inline":
                    if pieces and t == 2 and q == 0:
                        while pieces:
                            pieces.pop(0)[1]()
                elif pieces and q == 1:
                    if pieces[0][0] != "pj" or allow_pj:
                        pieces.pop(0)[1]()
                if probe != "half_exp" or q == 0:
                    nc.scalar.activation(
                        out=p_sb[:, q, :],
                        in_=s_ps,
                        func=mybir.ActivationFunctionType.Exp,
                        accum_out=(None if probe == "no_accum"
                                   else r_sb[:, mi, J:J + 1]),
                    )
                if pend is not None:
                    emit_attnv(pend, mi0)
                pend = (p_sb, q, mi)
                # dedup work in this exp's shadow (superblock 1 only):
                # transpose/evac/row-sum ops for rounds 4..7, then the
                # transposed tiles' attn@V for rounds 8..11
                if dedup and J == 1:
                    s = 2 * (t - 4) + q
                    if 0 <= s < 16:
                        emit_xq(s)
                    sj = 2 * (t - 12) + q
                    if 0 <= sj < 8:
                        for c in range(2):
                            nc.tensor.matmul(
                                xxp[c],
                                vt_sb[:, sj, :],
                                pT_sb[:, sj, ts(c, NBLK)],
                                start=False,
                                stop=False,
                            )
                # lookahead V^T / K-proj chunks (superblock 0 only), split
                # across the two exp shadows of the round
                if J == 0 and q == 0:
                    if t == 0:
                        emit_vt(0)
                        emit_vt(1)
                    if t < MT // 2 - 1:
                        emit_vt(2 * t + 2)
                        emit_vt(2 * t + 3)
                if J == 0 and q == 1 and t < 6:
                    emit_kproj(2 + t)
                # prefetch hook: emit the NEXT rep's prologue (input DMAs +
                # first K-proj chunks) into this rep's tail shadows
                if hook is not None and J == 1 and t == 10 and q == 0:
                    hook()
                    hook = None
            # queue the previous superblock's epilogue pieces
            if prev is not None and t == (5 if (dedup and J == 1) else 1):
                pieces.extend(epilogue_pieces(prev))
                prev = None
        if dedup and J == 1:
            # r2 write-out: c=1 chain copy + ship (c=0 copied in emit_xq)
            nc.vector.tensor_copy(r2_sb[:, ts(1, NBLK)], r2st["ps"])
            nc.sync.dma_start(out=r2_d.ap(), in_=r2_sb)
        emit_attnv(pend, mi0)  # flush: xx must close before this J's epilogue
        while pieces:  # anything not yet dripped
            pieces.pop(0)[1]()

        prev = {"J": J, "xx": xxp}

    # tail: epilogue for the last superblock + R write-out.  Returned as a
    # piece list so build_nc can defer it into the NEXT rep's emission
    # (cross-rep software pipelining); the final rep's pieces are emitted
    # at the program end with ACT relus (ACT idle there).
    last = prev

    def make_tail(tail):
        def r_piece():
            nc.sync.dma_start(out=r_d.ap(), in_=r_sb)
        return [("relu", r_piece)] + epilogue_pieces(last, tail=tail)

    return make_tail


def build_nc(reps=1, pack_s=True, probe=None):
    key = ("nc", reps, pack_s, probe)
    if key in _CACHE:
        return _CACHE[key]

    nc = bacc.Bacc("TRN2", target_bir_lowering=False, debug=False)

    xf_d = nc.dram_tensor("xf", [128, 2, N], F32R, kind="ExternalInput")
    xb_d = nc.dram_tensor("xb", [128, 2, N], BF16, kind="ExternalInput")
    wqkT_d = nc.dram_tensor("wqkT", [128, 2, 128], F32R, kind="ExternalInput")
    wvT_d = nc.dram_tensor("wvT", [128, 2, DH], BF16, kind="ExternalInput")
    wpT_d = nc.dram_tensor("wpT", [128, 2, 128], F32R, kind="ExternalInput")
    tqk_d = nc.dram_tensor("tqk", [128, 1], F32, kind="ExternalInput")
    tv_d = nc.dram_tensor("tv", [1, DH], F32, kind="ExternalInput")
    out_d = nc.dram_tensor("out", [2, 128, NSH], F32, kind="ExternalOutput")
    r_d = nc.dram_tensor("r", [128, MT, NSBLOCKS], F32, kind="ExternalOutput")
    ident_d = nc.dram_tensor("ident", [128, 128], BF16, kind="ExternalInput")
    r2_d = nc.dram_tensor("r2", [1, NSB], F32, kind="ExternalOutput")
    dram = (xf_d, xb_d, wqkT_d, wvT_d, wpT_d, tqk_d, tv_d, out_d, r_d,
            ident_d, r2_d)

    with tile.TileContext(nc) as tc:
        with (
            tc.tile_pool(name="const", bufs=1) as const,
            tc.tile_pool(name="work", bufs=3) as work,
            tc.tile_pool(name="outp", bufs=6) as outp,
            tc.tile_pool(name="pgrp", bufs=6) as pgrp,
            tc.tile_pool(name="psrc", bufs=4) as psrc,
            tc.tile_pool(name="ps_s", bufs=2, space="PSUM") as ps_s,
            tc.tile_pool(name="ps_xx", bufs=2, space="PSUM") as ps_xx,
            tc.tile_pool(name="ps_pj", bufs=1, space="PSUM") as ps_pj,
            tc.tile_pool(name="ps_t", bufs=1, space="PSUM") as ps_t,
        ):
            pools = (const, work, outp, pgrp, psrc, ps_s, ps_xx, ps_pj,
                     ps_t)
            make_tail = None
            ctx = _prologue(nc, pools, dram, probe=probe, rep=0)
            for rep in range(reps):
                holder = {}

                def hook(h=holder, nr=rep + 1):
                    if nr < reps:
                        h["ctx"] = _prologue(nc, pools, dram, probe=probe,
                                             rep=nr)

                if probe == "inline":
                    deferred = None
                else:
                    deferred = make_tail(False) if make_tail is not None else None
                make_tail = _emit(nc, pools, dram, ctx, pack_s=pack_s,
                                  probe=probe, rep=rep, deferred=deferred,
                                  hook=None if probe in ("nohook", "prebuf")
                                  else hook)
                if probe in ("nohook", "prebuf"):
                    if rep + 1 < reps:
                        holder["ctx"] = _prologue(nc, pools, dram,
                                                  probe=probe, rep=rep + 1)
                ctx = holder.get("ctx")
                if probe == "inline":
                    for _, piece in make_tail(True):
                        piece()
                    make_tail = None
            if make_tail is not None:
                for _, piece in make_tail(True):
                    piece()

    nc.compile()
    _CACHE[key] = nc
    return nc


def fold_bn(w, g, b, m, v):
    s = (g / np.sqrt(v + EPS)).astype(np.float32)
    return (w * s[:, None]).astype(np.float32), (b - m * s).astype(np.float32)


def make_in_maps(x, w_qk, g_qk, b_qk, m_qk, v_qk,
                 w_v, g_v, b_v, m_v, v_v, w_p, g_p, b_p, m_p, v_p):
    wqk_f, tqk_f = fold_bn(w_qk, g_qk, b_qk, m_qk, v_qk)   # [16,256], [16]
    wv_f, tv_f = fold_bn(w_v, g_v, b_v, m_v, v_v)          # [128,256], [128]
    wp_f, tp_f = fold_bn(w_p, g_p, b_p, m_p, v_p)          # [256,128], [256]

    # [128, 2, *]: partition dim first, C-half (or out-half) second.
    # wqkT replicated into 4 column groups of 32 (16 used + 16 zero) so the
    # S stage can row-pack 4 concurrent matmuls.
    wqkT_h = wqk_f.T.reshape(2, 128, KD).transpose(1, 0, 2)  # [128, 2, 16]
    wqkT = np.zeros((128, 2, 128), np.float32)
    for g in range(4):
        wqkT[:, :, 32 * g:32 * g + KD] = wqkT_h
    wqkT = np.ascontiguousarray(wqkT)
    wvT = np.ascontiguousarray(
        wv_f.T.reshape(2, 128, DH).transpose(1, 0, 2)).astype(ml_dtypes.bfloat16)
    wpT = np.ascontiguousarray(
        wp_f.T.reshape(128, 2, 128)).astype(np.float32)
    tqk = np.zeros((128, 1), np.float32)
    for g in range(4):
        tqk[32 * g:32 * g + KD, 0] = tqk_f
    tqk = np.ascontiguousarray(tqk)
    tv = tv_f.reshape(1, DH).astype(np.float32)

    xr = x.reshape(B, C, N).astype(np.float32)
    in_maps = []
    for c in range(N_CORES):
        b_, h_ = c // 2, c % 2
        # permute n so this core's half comes first
        if h_ == 0:
            xp = xr[b_]
        else:
            xp = np.concatenate([xr[b_][:, NSH:], xr[b_][:, :NSH]], axis=1)
        xp = np.ascontiguousarray(xp.reshape(2, 128, N).transpose(1, 0, 2))
        in_maps.append({
            "xf": xp.astype(np.float32),
            "xb": xp.astype(ml_dtypes.bfloat16),
            "wqkT": wqkT, "wvT": wvT, "wpT": wpT,
            "tqk": tqk, "tv": tv,
            "ident": np.eye(128, dtype=ml_dtypes.bfloat16),
        })
    return in_maps, tp_f


def assemble(results, tp_f):
    """Per-core 'out' [2,128,NSH] + 'r' [128,MT,NSBLOCKS] -> full [B,C,H,W].

    r[n] = R_half0[n] + R_half1[n] by symmetry of P; final epilogue
    out = O_u / r + t_p (relu commutes with the positive 1/r scale, so the
    device applied w_p' @ relu() unnormalized).
    """
    out = np.empty((B, C, N), np.float32)
    for b_ in range(B):
        rv = []
        for h_ in range(2):
            res = results[2 * b_ + h_]
            rr = res["r"].astype(np.float32)  # [128,MT,2]
            # m-tiles 0..7 skip the J=1 exp (their P comes from the diag
            # transpose); that part of R arrives as the separate r2 vector
            rr[:, 0:8, 1] = 0.0
            rl = rr.sum(axis=2).T.reshape(-1)  # R[m], local (permuted) order
            rl[0:NSB] += res["r2"].reshape(-1)
            if h_ == 1:
                rl = np.concatenate([rl[NSH:], rl[:NSH]])      # unpermute
            rv.append(rl)
        r = rv[0] + rv[1]                                      # [N]
        for h_ in range(2):
            o = results[2 * b_ + h_]["out"].reshape(C, NSH)
            sl = slice(h_ * NSH, (h_ + 1) * NSH)
            out[b_][:, sl] = o / r[sl][None, :] + tp_f[:, None]
    return out.reshape(B, C, H, W)


def kernel(**inputs):
    from concourse.bass_utils import run_bass_kernel_spmd
    from concourse.bass_interp import get_hw_module

    inputs = {k: np.asarray(v) for k, v in inputs.items()}
    inputs.pop("key_v_input_reduction", None)  # unused by the reference
    nc = build_nc()
    in_maps, tp_f = make_in_maps(**inputs)
    old_m = nc.m
    nc.m = get_hw_module(nc.m)
    try:
        res = run_bass_kernel_spmd(nc, in_maps, core_ids=list(range(N_CORES)))
    finally:
        nc.m = old_m
    return assemble(res.results, tp_f)
